# revision 1
# baseline (speedup 1.0000x reference)
"""Causal self-attention with RoPE on 8 Trainium2 NeuronCores.

Problem: B=4, S=4096, E=64, H=4 heads x D=16, fp32 in/out.

Sharding: core c handles batch b = c//2 and head-pair hp = c%2 (heads 2*hp,
2*hp+1).  Every core runs the IDENTICAL program (SPMD) -- per-core behavior
comes only from the data (x[b] and per-head weight slices).  Each core
returns the partial output projection sum over its two heads; the host adds
the two partials per batch.

Device algorithm (per core, per head):
  - x^T [64,S] via PE transposes (bf16)
  - K^T,Q^T projections as lhsT.T@x^T (scale 1/sqrt(D) folded into Wq);
    RoPE applied as  rot = proj * cos + proj_shuf * sin  where proj_shuf
    comes from a sign/permuted weight matrix (R@W) -- no cross-partition ops
  - scores computed TRANSPOSED: S^T[k',q] = K^T.T@... (contraction over d=16)
    so softmax normalization folds into the PE: V is augmented with a ones
    column, attended^T[17,q] accumulates over k'-tiles with row 16 = the
    softmax denominator.  Unstable softmax (no max subtraction) is safe here:
    scores ~ N(0,1).
  - causal mask applied post-exp with affine_select (fill 0)
  - normalize with reciprocal + gpsimd partition_broadcast + multiply
  - output projection accumulated over the 2 heads, PE-transposed back to
    row-major, DMA out.
"""

import sys

sys.path.insert(0, "/opt/trn_rl_repo")

import numpy as np
import ml_dtypes

B, S, E, H, D = 4, 4096, 64, 4, 16
NCORES = 8
NKT = S // 128  # 32 k-tiles of 128
NQC = S // 512  # 8 q-chunks of 512
KT_GROUP = 2    # k-tiles per exp batch (2 PSUM banks)

BF16 = ml_dtypes.bfloat16

_CACHE: dict = {}


def _rope_tables():
    # cos/sin[16*hh + d, s] = cos/sin(s * invfreq[d//2]); same for both heads
    pos = np.arange(S, dtype=np.float64)
    pair = np.arange(0, D, 2, dtype=np.float64)  # 0,2,..,14
    inv = 1.0 / (10000.0 ** (pair / D))          # [8]
    ang = pos[None, :] * inv[:, None]            # [8, S]
    cos8, sin8 = np.cos(ang), np.sin(ang)
    cos16 = np.repeat(cos8, 2, axis=0)           # [16, S] rows 2p,2p+1 equal
    sin16 = np.repeat(sin8, 2, axis=0)
    cos32 = np.concatenate([cos16, cos16], axis=0)  # [32, S] both heads
    sin32 = np.concatenate([sin16, sin16], axis=0)
    return cos32.astype(BF16), sin32.astype(BF16)


def _shuffle_rows(w):
    # (R w)[2p] = -w[2p+1], (R w)[2p+1] = w[2p]   (rope partner)
    ws = np.empty_like(w)
    ws[0::2] = -w[1::2]
    ws[1::2] = w[0::2]
    return ws


def make_core_inputs(x, Wq, Wk, Wv, Wo, core):
    """Build the per-core input map (all host-side numpy)."""
    b, hp = core // 2, core % 2
    rs = slice(32 * hp, 32 * hp + 32)  # rows of the 2 heads in W{q,k,v}
    scale = 1.0 / np.sqrt(np.float32(D))

    wq_sel = (Wq[rs] * scale).astype(np.float32)  # [32, 64]
    wk_sel = Wk[rs].astype(np.float32)
    cos32, sin32 = _CACHE.setdefault("rope", _rope_tables())

    def gap48(w32):
        # [32,64] head rows -> [64,48] lhsT with head hh at cols 32*hh+0:16
        out = np.zeros((64, 48), np.float32)
        out[:, 0:16] = w32[0:16].T
        out[:, 32:48] = w32[16:32].T
        return out

    def gap48t(t32):
        out = np.zeros((48, t32.shape[1]), t32.dtype)
        out[0:16] = t32[0:16]
        out[32:48] = t32[16:32]
        return out

    return {
        "x": np.ascontiguousarray(x[b]).astype(BF16),
        "wq": np.ascontiguousarray(gap48(wq_sel)).astype(BF16),       # [64,48]
        "wk": np.ascontiguousarray(gap48(wk_sel)).astype(BF16),
        "wqs": np.ascontiguousarray(gap48(_shuffle_rows(wq_sel))).astype(BF16),
        "wks": np.ascontiguousarray(gap48(_shuffle_rows(wk_sel))).astype(BF16),
        "wv": np.ascontiguousarray(Wv[rs].T).astype(BF16),            # [64,32]
        # wo[d, hh, e] = Wo[e, 16*(2hp+hh)+d]
        "wo": np.ascontiguousarray(
            Wo[:, rs].reshape(E, 2, D).transpose(2, 1, 0)
        ).astype(BF16),                                               # [16,2,64]
        "cost": gap48t(cos32),
        "sint": gap48t(sin32),
        "idf": np.eye(128, dtype=np.float32),
        "idb": np.eye(64, dtype=BF16),
    }


def partial_reference(inp):
    """Numpy reference of ONE core's partial output (for testing)."""
    x = inp["x"].astype(np.float64)
    cos = inp["cost"].astype(np.float64)[0:16]
    sin = inp["sint"].astype(np.float64)[0:16]
    out = np.zeros((S, E))
    for hh in range(2):
        wq = inp["wq"].astype(np.float64)[:, 32 * hh : 32 * hh + 16]
        wqs = inp["wqs"].astype(np.float64)[:, 32 * hh : 32 * hh + 16]
        wk = inp["wk"].astype(np.float64)[:, 32 * hh : 32 * hh + 16]
        wks = inp["wks"].astype(np.float64)[:, 32 * hh : 32 * hh + 16]
        wv = inp["wv"].astype(np.float64)[:, 16 * hh : 16 * hh + 16]
        wo = inp["wo"].astype(np.float64)[:, hh, :]  # [16, 64]
        q = (x @ wq) * cos.T + (x @ wqs) * sin.T     # [S,16]
        k = (x @ wk) * cos.T + (x @ wks) * sin.T
        v = x @ wv
        s = q @ k.T
        mask = np.tril(np.ones((S, S), dtype=bool))
        p = np.where(mask, np.exp(s), 0.0)
        a = (p @ v) / p.sum(-1, keepdims=True)       # [S,16]
        out += a @ wo
    return out.astype(np.float32)


def build_nc(probe=None, amp=1, split_waits=True, ablate=None):
    """Build the (single, SPMD) Bass program.

    Pipeline: one fused loop over ci = 0..7.  Iteration ci builds x^T chunk
    ci (PE transposes), projects+ropes K^T/Q^T chunk ci, builds V k-tiles
    4ci..4ci+3, then runs causal attention for query chunk qc=ci (which only
    needs K/V up to k-tile 4ci+3).  Projection work for ci+1 gap-fills PE
    stalls during attention of qc=ci.
    """
    import os
    ablate = ablate or os.environ.get("KABLATE") or ()
    import concourse.bass as bass
    import concourse.mybir as mybir
    import concourse.tile as tile

    f32 = mybir.dt.float32
    bf16 = mybir.dt.bfloat16
    AF = mybir.ActivationFunctionType
    OP = mybir.AluOpType

    nc = bass.Bass()
    x_d = nc.declare_dram_parameter("x", [S, E], bf16, isOutput=False)
    wq_d = nc.declare_dram_parameter("wq", [E, 48], bf16, isOutput=False)
    wk_d = nc.declare_dram_parameter("wk", [E, 48], bf16, isOutput=False)
    wqs_d = nc.declare_dram_parameter("wqs", [E, 48], bf16, isOutput=False)
    wks_d = nc.declare_dram_parameter("wks", [E, 48], bf16, isOutput=False)
    wv_d = nc.declare_dram_parameter("wv", [E, 32], bf16, isOutput=False)
    wo_d = nc.declare_dram_parameter("wo", [D, 2, E], bf16, isOutput=False)
    cos_d = nc.declare_dram_parameter("cost", [48, S], bf16, isOutput=False)
    sin_d = nc.declare_dram_parameter("sint", [48, S], bf16, isOutput=False)
    idf_d = nc.declare_dram_parameter("idf", [128, 128], f32, isOutput=False)
    idb_d = nc.declare_dram_parameter("idb", [64, 64], bf16, isOutput=False)
    out_d = nc.declare_dram_parameter("out", [S, E], f32, isOutput=True)
    # DRAM scratch for the denominator partition-broadcast (DMA bounce)
    scr_d = nc.dram_tensor("nrm_scratch", [2 * NQC, 512], f32)

    with tile.TileContext(nc) as tc:
        with tc.tile_pool(name="persist", bufs=1) as pp:
            # ---- constants into SBUF ----
            wq_sb = pp.tile([E, 48], bf16, name="wq_sb")
            wk_sb = pp.tile([E, 48], bf16, name="wk_sb")
            wqs_sb = pp.tile([E, 48], bf16, name="wqs_sb")
            wks_sb = pp.tile([E, 48], bf16, name="wks_sb")
            wv_sb = pp.tile([E, 32], bf16, name="wv_sb")
            cos_sb = pp.tile([48, S], bf16, name="cos_sb")
            sin_sb = pp.tile([48, S], bf16, name="sin_sb")
            idb_sb = pp.tile([64, 64], bf16, name="idb_sb")
            for sb, dr in [
                (wq_sb, wq_d), (wk_sb, wk_d), (wqs_sb, wqs_d), (wks_sb, wks_d),
                (wv_sb, wv_d), (cos_sb, cos_d), (sin_sb, sin_d),
                (idb_sb, idb_d),
            ]:
                nc.sync.dma_start(sb, dr[:])
            # wo per head at partitions 64*hh (so the out-projection lhsT
            # shares the contraction partition range with an[64*hh:...])
            wo_sb = pp.tile([128, E], bf16, name="wo_sb")
            for hh in range(2):
                nc.sync.dma_start(wo_sb[64 * hh : 64 * hh + D, :], wo_d[:, hh, :])

            # ---- persistent activations ----
            xT = pp.tile([E, S], bf16, name="xT")
            nc.sync.dma_start_transpose(xT, x_d[:, :])
            rotK = pp.tile([48, S], bf16, name="rotK")
            rotQ = pp.tile([48, S], bf16, name="rotQ")
            vp = pp.tile([128, NKT, 2, 33], bf16, name="vp")
            nc.vector.memset(vp, 0.0)
            nc.vector.memset(vp[:, :, :, 32:33], 1.0)

            with tc.tile_pool(name="a_pr", bufs=1, space="PSUM") as pr, \
                 tc.tile_pool(name="a_ps", bufs=1, space="PSUM") as sp, \
                 tc.tile_pool(name="a_att", bufs=1, space="PSUM") as ap_, \
                 tc.tile_pool(name="a_po", bufs=1, space="PSUM") as op_, \
                 tc.tile_pool(name="a_p", bufs=3) as pb_, \
                 tc.tile_pool(name="a_n", bufs=2) as nb, \
                 tc.tile_pool(name="a_f", bufs=2) as fb:
              for _rep in range(amp):
                for ci in range(NQC + 1):
                  if ci < NQC:
                    cs = slice(512 * ci, 512 * (ci + 1))
                    # -- projections + rope, chunk ci --
                    for wmat, wsh, rot, nm in (
                        (wk_sb, wks_sb, rotK, "k"),
                        (wq_sb, wqs_sb, rotQ, "q"),
                    ):
                        pa = pr.tile([48, 512], f32, tag="pa", name=f"pa{nm}")
                        nc.tensor.matmul(pa, wmat, xT[:, cs], start=True, stop=True)
                        pb = pr.tile([48, 512], f32, tag="pb", name=f"pb{nm}")
                        nc.tensor.matmul(pb, wsh, xT[:, cs], start=True, stop=True)
                        t1 = nb.tile([48, 512], bf16, tag=f"t1{nm}", name=f"t1{nm}")
                        nc.vector.tensor_tensor(t1, pa, cos_sb[:, cs], OP.mult)
                        t2 = nb.tile([48, 512], bf16, tag=f"t2{nm}", name=f"t2{nm}")
                        nc.vector.tensor_tensor(t2, pb, sin_sb[:, cs], OP.mult)
                        nc.vector.tensor_tensor(rot[:, cs], t1, t2, OP.add)
                    # -- V k-tiles of chunk ci --
                    for ii in range(4):
                        i = 4 * ci + ii
                        pv = pr.tile([128, 32], f32, tag=("pa", "pb")[ii % 2],
                                     name="pv")
                        nc.tensor.matmul(
                            pv, xT[:, 128 * i : 128 * (i + 1)], wv_sb,
                            start=True, stop=True,
                        )
                        nc.vector.tensor_copy(
                            vp[:, i, :, 0:D],
                            pv.rearrange("p (h d) -> p h d", h=2),
                        )
                  # -- causal attention, one chunk behind projection --
                  qc = ci - 1
                  if qc >= 0:
                    nk = 4 * qc + 4
                    qs = slice(512 * qc, 512 * (qc + 1))
                    po = op_.tile([E, 512], f32, tag="po", name="po")
                    att = ap_.tile([128, 512], f32, tag="att", name="att")
                    # both heads accumulate in one bank (rows 64*hh..+33).
                    # start=True would clear the whole bank's has_written
                    # bits and corrupt the other head's region, so memset
                    # the values once and accumulate with start=False
                    # (accumulate-onto-0 == overwrite, either bit state).
                    nc.vector.memset(att, 0.0)
                    def emit_att(kts_, pts_):
                        for j, kt in enumerate(kts_):
                            for hh in range(2):
                                nc.tensor.matmul(
                                    att[64 * hh : 64 * hh + 33, :],
                                    vp[:, kt, hh, :], pts_[hh][:, j, :],
                                    start=False, stop=(kt == nk - 1),
                                    skip_group_check=True,
                                    tile_position=(0, 64 * hh),
                                )

                    prev = None
                    for g0 in range(0, nk, KT_GROUP):
                        kts = list(range(g0, min(g0 + KT_GROUP, nk)))
                        pss, pts = [], []
                        for hh in range(2):
                            beta = 32 * hh
                            ps = sp.tile([128, KT_GROUP, 512], f32,
                                         tag=f"s{hh}", name=f"ps{hh}")
                            pss.append(ps)
                            for j, kt in enumerate(kts):
                                nc.tensor.matmul(
                                    ps[:, j, :],
                                    rotK[beta : beta + D,
                                         128 * kt : 128 * (kt + 1)],
                                    rotQ[beta : beta + D, qs],
                                    start=True, stop=True,
                                )
                        for hh in range(2):
                            pt = pb_.tile([128, KT_GROUP, 512], bf16,
                                          tag=f"p{hh}", name=f"pt{hh}")
                            pts.append(pt)
                            nc.scalar.activation(
                                pt[:, 0 : len(kts), :],
                                pss[hh][:, 0 : len(kts), :], AF.Exp,
                            )
                        for j, kt in enumerate(kts):
                            for hh in range(2):
                                r = 128 * kt - 512 * qc
                                if r >= 0 and "noaffine" not in ablate:
                                    nc.gpsimd.affine_select(
                                        out=pts[hh][:, j, :],
                                        in_=pts[hh][:, j, :],
                                        pattern=[[1, 512]],
                                        compare_op=mybir.AluOpType.is_ge,
                                        fill=0.0, base=-r,
                                        channel_multiplier=-1,
                                    )
                        # attended MMs one group late: PE never blocks on
                        # this group's exp -- it still has next scores ready
                        if prev is not None:
                            emit_att(*prev)
                        prev = (kts, pts)
                    emit_att(*prev)
                    for hh in range(2):
                        base = 64 * hh
                        rec = nb.tile([128, 512], f32, tag="rec", name="rec")
                        nc.vector.reciprocal(
                            rec[base + 32 : base + 33, :],
                            att[base + 32 : base + 33, :],
                        )
                        sidx = 2 * qc + hh
                        nc.sync.dma_start(
                            scr_d[sidx : sidx + 1, :],
                            rec[base + 32 : base + 33, :],
                        )
                        bc = nb.tile([128, 512], f32, tag="bc", name="bc")
                        nc.sync.dma_start(
                            bc[base : base + D, :],
                            scr_d[sidx : sidx + 1, :].broadcast_to([D, 512]),
                        )
                        an = nb.tile([128, 512], bf16, tag="an", name="an")
                        nc.vector.tensor_tensor(
                            an[base : base + D, :], att[base : base + D, :],
                            bc[base : base + D, :], OP.mult,
                        )
                        nc.tensor.matmul(
                            po, wo_sb[base : base + D, :],
                            an[base : base + D, :],
                            start=(hh == 0), stop=(hh == 1),
                        )
                    # transpose out^T [64,512] back to row-major and store
                    oT = fb.tile([E, 512], bf16, tag="oT", name="oT")
                    nc.vector.tensor_copy(oT, po)
                    tr = ap_.tile([128, 4, E], bf16, tag="att", name="tr")
                    for i in range(4):
                        nc.tensor.matmul(
                            tr[:, i, :], oT[:, 128 * i : 128 * (i + 1)], idb_sb,
                            is_transpose=True, start=True, stop=True,
                        )
                    ob = fb.tile([128, 4, E], f32, tag="ob", name="ob")
                    nc.vector.tensor_copy(ob, tr)
                    nc.sync.dma_start(
                        out_d[qs, :].rearrange("(c p) e -> p c e", p=128), ob
                    )
    # populate .instr bytes for extended-inst InstISA subclasses (raw Bass
    # does not run this pass; without it walrus fails "ISA wrong length")
    from concourse.library_overlay import lower_extended_insts
    lower_extended_insts(nc)
    if split_waits:  # required for walrus; breaks CoreSim's race detector
        _split_multi_waits(nc, mybir)
    return nc


def _split_multi_waits(nc, mybir):
    """This walrus build accepts at most ONE sync-wait command per
    instruction ("Too many sync wait commands").  Tile emits instructions
    with several waits; hoist all but the last into standalone
    InstEventSemaphore (sequencer wait) instructions on the same engine,
    inserted immediately before."""
    import bass_rust

    uid = [0]
    for f in nc.m.functions:
        for blk in f.blocks:
            insts = list(blk.instructions)
            out = []
            changed = False
            for inst in insts:
                si = inst.sync_info
                waits = list(si.on_wait) if si is not None else []
                if len(waits) > 1:
                    changed = True
                    for w in waits[:-1]:
                        ev = mybir.InstEventSemaphore(
                            name=f"WSPLIT-{uid[0]}", ins=[], outs=[]
                        )
                        uid[0] += 1
                        ev.engine = inst.engine
                        ev.sync_info = bass_rust.SyncInfo(
                            on_wait=[w], on_update=[]
                        )
                        out.append(ev)
                    inst.sync_info = bass_rust.SyncInfo(
                        on_wait=[waits[-1]], on_update=list(si.on_update)
                    )
                out.append(inst)
            if changed:
                blk.instructions = out


def _get_nc(probe=None):
    key = ("nc", probe)
    if key not in _CACHE:
        _CACHE[key] = build_nc(probe)
    return _CACHE[key]


def kernel(x, Wq, Wk, Wv, Wo):
    from concourse.bass_utils import run_bass_kernel_spmd

    x = np.asarray(x, dtype=np.float32)
    Wq, Wk, Wv, Wo = (np.asarray(w, dtype=np.float32) for w in (Wq, Wk, Wv, Wo))

    nc = _get_nc()
    in_maps = [make_core_inputs(x, Wq, Wk, Wv, Wo, c) for c in range(NCORES)]
    res = run_bass_kernel_spmd(nc, in_maps, core_ids=list(range(NCORES)))
    out = np.empty((B, S, E), dtype=np.float32)
    for b in range(B):
        out[b] = res.results[2 * b]["out"] + res.results[2 * b + 1]["out"]
    return out



# revision 11
# speedup vs baseline: 1.8479x; 1.8479x over previous
"""Causal self-attention with RoPE on 8 Trainium2 NeuronCores.

Problem: B=4, S=4096, E=64, H=4 heads x D=16, fp32 in/out.

Sharding: core c handles batch b = c//2 and head-pair hp = c%2 (heads 2*hp,
2*hp+1).  Every core runs the IDENTICAL program (SPMD) -- per-core behavior
comes only from the data (x[b] and per-head weight slices).  Each core
returns the partial output projection sum over its two heads; the host adds
the two partials per batch.

Device algorithm (per core, per head):
  - x^T [64,S] via PE transposes (bf16)
  - K^T,Q^T projections as lhsT.T@x^T (scale 1/sqrt(D) folded into Wq);
    RoPE applied as  rot = proj * cos + proj_shuf * sin  where proj_shuf
    comes from a sign/permuted weight matrix (R@W) -- no cross-partition ops
  - scores computed TRANSPOSED: S^T[k',q] = K^T.T@... (contraction over d=16)
    so softmax normalization folds into the PE: V is augmented with a ones
    column, attended^T[17,q] accumulates over k'-tiles with row 16 = the
    softmax denominator.  Unstable softmax (no max subtraction) is safe here:
    scores ~ N(0,1).
  - causal mask applied post-exp with affine_select (fill 0)
  - normalize with reciprocal + gpsimd partition_broadcast + multiply
  - output projection accumulated over the 2 heads, PE-transposed back to
    row-major, DMA out.
"""

import sys

sys.path.insert(0, "/opt/trn_rl_repo")

import numpy as np
import ml_dtypes

B, S, E, H, D = 4, 4096, 64, 4, 16
NCORES = 8
NKT = S // 128  # 32 k-tiles of 128
NQC = S // 512  # 8 q-chunks of 512
KT_GROUP = 2    # k-tiles per exp batch (2 PSUM banks)

BF16 = ml_dtypes.bfloat16

_CACHE: dict = {}


def _rope_tables():
    # cos/sin[16*hh + d, s] = cos/sin(s * invfreq[d//2]); same for both heads
    pos = np.arange(S, dtype=np.float64)
    pair = np.arange(0, D, 2, dtype=np.float64)  # 0,2,..,14
    inv = 1.0 / (10000.0 ** (pair / D))          # [8]
    ang = pos[None, :] * inv[:, None]            # [8, S]
    cos8, sin8 = np.cos(ang), np.sin(ang)
    cos16 = np.repeat(cos8, 2, axis=0)           # [16, S] rows 2p,2p+1 equal
    sin16 = np.repeat(sin8, 2, axis=0)
    cos32 = np.concatenate([cos16, cos16], axis=0)  # [32, S] both heads
    sin32 = np.concatenate([sin16, sin16], axis=0)
    return cos32.astype(BF16), sin32.astype(BF16)


def _shuffle_rows(w):
    # (R w)[2p] = -w[2p+1], (R w)[2p+1] = w[2p]   (rope partner)
    ws = np.empty_like(w)
    ws[0::2] = -w[1::2]
    ws[1::2] = w[0::2]
    return ws


def make_core_inputs(x, Wq, Wk, Wv, Wo, core):
    """Build the per-core input map (all host-side numpy)."""
    b, hp = core // 2, core % 2
    rs = slice(32 * hp, 32 * hp + 32)  # rows of the 2 heads in W{q,k,v}
    scale = 1.0 / np.sqrt(np.float32(D))

    wq_sel = (Wq[rs] * scale).astype(np.float32)  # [32, 64]
    wk_sel = Wk[rs].astype(np.float32)
    cos32, sin32 = _CACHE.setdefault("rope", _rope_tables())

    def gap48(w32):
        # [32,64] head rows -> [64,48] lhsT with head hh at cols 32*hh+0:16
        out = np.zeros((64, 48), np.float32)
        out[:, 0:16] = w32[0:16].T
        out[:, 32:48] = w32[16:32].T
        return out

    def gap48t(t32):
        out = np.zeros((48, t32.shape[1]), t32.dtype)
        out[0:16] = t32[0:16]
        out[32:48] = t32[16:32]
        return out

    return {
        "xt": np.ascontiguousarray(x[b].T).astype(BF16),              # [64,S]
        "wq": np.ascontiguousarray(gap48(wq_sel)).astype(BF16),       # [64,48]
        "wk": np.ascontiguousarray(gap48(wk_sel)).astype(BF16),
        "wqs": np.ascontiguousarray(gap48(_shuffle_rows(wq_sel))).astype(BF16),
        "wks": np.ascontiguousarray(gap48(_shuffle_rows(wk_sel))).astype(BF16),
        "wv": np.ascontiguousarray(Wv[rs].T).astype(BF16),            # [64,32]
        # wo[d, hh, e] = Wo[e, 16*(2hp+hh)+d]
        "wo": np.ascontiguousarray(
            Wo[:, rs].reshape(E, 2, D).transpose(2, 1, 0)
        ).astype(BF16),                                               # [16,2,64]
        "cost": gap48t(cos32),
        "sint": gap48t(sin32),
        "idb": np.eye(64, dtype=BF16),
    }


def partial_reference(inp):
    """Numpy reference of ONE core's partial output (for testing)."""
    x = inp["xt"].astype(np.float64).T
    cos = inp["cost"].astype(np.float64)[0:16]
    sin = inp["sint"].astype(np.float64)[0:16]
    out = np.zeros((S, E))
    for hh in range(2):
        wq = inp["wq"].astype(np.float64)[:, 32 * hh : 32 * hh + 16]
        wqs = inp["wqs"].astype(np.float64)[:, 32 * hh : 32 * hh + 16]
        wk = inp["wk"].astype(np.float64)[:, 32 * hh : 32 * hh + 16]
        wks = inp["wks"].astype(np.float64)[:, 32 * hh : 32 * hh + 16]
        wv = inp["wv"].astype(np.float64)[:, 16 * hh : 16 * hh + 16]
        wo = inp["wo"].astype(np.float64)[:, hh, :]  # [16, 64]
        q = (x @ wq) * cos.T + (x @ wqs) * sin.T     # [S,16]
        k = (x @ wk) * cos.T + (x @ wks) * sin.T
        v = x @ wv
        s = q @ k.T
        mask = np.tril(np.ones((S, S), dtype=bool))
        p = np.where(mask, np.exp(s), 0.0)
        a = (p @ v) / p.sum(-1, keepdims=True)       # [S,16]
        out += a @ wo
    return out.astype(np.float32)


def build_nc(probe=None, amp=1, split_waits=True, ablate=None):
    """Build the (single, SPMD) Bass program.

    Pipeline: one fused loop over ci = 0..7.  Iteration ci builds x^T chunk
    ci (PE transposes), projects+ropes K^T/Q^T chunk ci, builds V k-tiles
    4ci..4ci+3, then runs causal attention for query chunk qc=ci (which only
    needs K/V up to k-tile 4ci+3).  Projection work for ci+1 gap-fills PE
    stalls during attention of qc=ci.
    """
    import os
    ablate = ablate or os.environ.get("KABLATE") or ()
    import concourse.bass as bass
    import concourse.mybir as mybir
    import concourse.tile as tile

    f32 = mybir.dt.float32
    bf16 = mybir.dt.bfloat16
    AF = mybir.ActivationFunctionType
    OP = mybir.AluOpType

    nc = bass.Bass()
    xt_d = nc.declare_dram_parameter("xt", [E, S], bf16, isOutput=False)
    wq_d = nc.declare_dram_parameter("wq", [E, 48], bf16, isOutput=False)
    wk_d = nc.declare_dram_parameter("wk", [E, 48], bf16, isOutput=False)
    wqs_d = nc.declare_dram_parameter("wqs", [E, 48], bf16, isOutput=False)
    wks_d = nc.declare_dram_parameter("wks", [E, 48], bf16, isOutput=False)
    wv_d = nc.declare_dram_parameter("wv", [E, 32], bf16, isOutput=False)
    wo_d = nc.declare_dram_parameter("wo", [D, 2, E], bf16, isOutput=False)
    cos_d = nc.declare_dram_parameter("cost", [48, S], bf16, isOutput=False)
    sin_d = nc.declare_dram_parameter("sint", [48, S], bf16, isOutput=False)
    idb_d = nc.declare_dram_parameter("idb", [64, 64], bf16, isOutput=False)
    out_d = nc.declare_dram_parameter("out", [S, E], f32, isOutput=True)
    # DRAM scratch for the denominator partition-broadcast (DMA bounce)
    scr_d = nc.dram_tensor("nrm_scratch", [2 * NQC, 512], f32)

    with tile.TileContext(nc) as tc:
        with tc.tile_pool(name="persist", bufs=1) as pp:
            # ---- constants into SBUF ----
            wq_sb = pp.tile([E, 48], bf16, name="wq_sb")
            wk_sb = pp.tile([E, 48], bf16, name="wk_sb")
            wqs_sb = pp.tile([E, 48], bf16, name="wqs_sb")
            wks_sb = pp.tile([E, 48], bf16, name="wks_sb")
            wv_sb = pp.tile([E, 32], bf16, name="wv_sb")
            cos_sb = pp.tile([48, S], bf16, name="cos_sb")
            sin_sb = pp.tile([48, S], bf16, name="sin_sb")
            idb_sb = pp.tile([64, 64], bf16, name="idb_sb")
            for sb, dr in [
                (wq_sb, wq_d), (wk_sb, wk_d), (wqs_sb, wqs_d), (wks_sb, wks_d),
                (wv_sb, wv_d), (cos_sb, cos_d), (sin_sb, sin_d),
                (idb_sb, idb_d),
            ]:
                nc.sync.dma_start(sb, dr[:])
            # wo per head at partitions 64*hh (so the out-projection lhsT
            # shares the contraction partition range with an[64*hh:...])
            wo_sb = pp.tile([128, E], bf16, name="wo_sb")
            for hh in range(2):
                nc.sync.dma_start(wo_sb[64 * hh : 64 * hh + D, :], wo_d[:, hh, :])

            # ---- persistent activations ----
            xT = pp.tile([E, S], bf16, name="xT")
            nc.sync.dma_start(xT, xt_d[:])
            rotK = pp.tile([48, S], bf16, name="rotK")
            rotQ = pp.tile([48, S], bf16, name="rotQ")
            vp = pp.tile([128, NKT, 2, 33], bf16, name="vp")
            nc.vector.memset(vp, 0.0)
            nc.vector.memset(vp[:, :, :, 32:33], 1.0)

            with tc.tile_pool(name="a_pr", bufs=1, space="PSUM") as pr, \
                 tc.tile_pool(name="a_ps", bufs=1, space="PSUM") as sp, \
                 tc.tile_pool(name="a_att", bufs=1, space="PSUM") as ap_, \
                 tc.tile_pool(name="a_po", bufs=1, space="PSUM") as op_, \
                 tc.tile_pool(name="a_p", bufs=3) as pb_, \
                 tc.tile_pool(name="a_n", bufs=2) as nb, \
                 tc.tile_pool(name="a_f", bufs=2) as fb:
              for _rep in range(amp):
                for ci in range(NQC + 1):
                  if ci < NQC:
                    cs = slice(512 * ci, 512 * (ci + 1))
                    # -- projections + rope, chunk ci --
                    for wmat, wsh, rot, nm in (
                        (wk_sb, wks_sb, rotK, "k"),
                        (wq_sb, wqs_sb, rotQ, "q"),
                    ):
                        pa = pr.tile([48, 512], f32, tag="pa", name=f"pa{nm}")
                        nc.tensor.matmul(pa, wmat, xT[:, cs], start=True, stop=True)
                        pb = pr.tile([48, 512], f32, tag="pb", name=f"pb{nm}")
                        nc.tensor.matmul(pb, wsh, xT[:, cs], start=True, stop=True)
                        t1 = nb.tile([48, 512], bf16, tag=f"t1{nm}", name=f"t1{nm}")
                        nc.vector.tensor_tensor(t1, pa, cos_sb[:, cs], OP.mult)
                        t2 = nb.tile([48, 512], bf16, tag=f"t2{nm}", name=f"t2{nm}")
                        nc.vector.tensor_tensor(t2, pb, sin_sb[:, cs], OP.mult)
                        nc.vector.tensor_tensor(rot[:, cs], t1, t2, OP.add)
                    # -- V k-tiles of chunk ci --
                    for ii in range(4):
                        i = 4 * ci + ii
                        pv = pr.tile([128, 32], f32, tag=("pa", "pb")[ii % 2],
                                     name="pv")
                        nc.tensor.matmul(
                            pv, xT[:, 128 * i : 128 * (i + 1)], wv_sb,
                            start=True, stop=True,
                        )
                        nc.vector.tensor_copy(
                            vp[:, i, :, 0:D],
                            pv.rearrange("p (h d) -> p h d", h=2),
                        )
                  # -- causal attention, one chunk behind projection --
                  qc = ci - 1
                  if qc >= 0:
                    nk = 4 * qc + 4
                    qs = slice(512 * qc, 512 * (qc + 1))
                    po = op_.tile([E, 512], f32, tag="po", name="po")
                    att = ap_.tile([128, 512], f32, tag="att", name="att")
                    # both heads accumulate in one bank (rows 64*hh..+33).
                    # start=True would clear the whole bank's has_written
                    # bits and corrupt the other head's region, so memset
                    # the values once and accumulate with start=False
                    # (accumulate-onto-0 == overwrite, either bit state).
                    nc.vector.memset(att, 0.0)
                    def emit_att(kts_, pts_):
                        for j, kt in enumerate(kts_):
                            for hh in range(2):
                                nc.tensor.matmul(
                                    att[64 * hh : 64 * hh + 33, :],
                                    vp[:, kt, hh, :], pts_[hh][:, j, :],
                                    start=False, stop=(kt == nk - 1),
                                    skip_group_check=True,
                                    tile_position=(0, 64 * hh),
                                )

                    prev = None
                    for g0 in range(0, nk, KT_GROUP):
                        kts = list(range(g0, min(g0 + KT_GROUP, nk)))
                        pss, pts = [], []
                        for hh in range(2):
                            beta = 32 * hh
                            ps = sp.tile([128, KT_GROUP, 512], f32,
                                         tag=f"s{hh}", name=f"ps{hh}")
                            pss.append(ps)
                            for j, kt in enumerate(kts):
                                nc.tensor.matmul(
                                    ps[:, j, :],
                                    rotK[beta : beta + D,
                                         128 * kt : 128 * (kt + 1)],
                                    rotQ[beta : beta + D, qs],
                                    start=True, stop=True,
                                )
                        for hh in range(2):
                            pt = pb_.tile([128, KT_GROUP, 512], bf16,
                                          tag=f"p{hh}", name=f"pt{hh}")
                            pts.append(pt)
                            nc.scalar.activation(
                                pt[:, 0 : len(kts), :],
                                pss[hh][:, 0 : len(kts), :], AF.Exp,
                            )
                        for j, kt in enumerate(kts):
                            for hh in range(2):
                                r = 128 * kt - 512 * qc
                                if r >= 0 and "noaffine" not in ablate:
                                    nc.gpsimd.affine_select(
                                        out=pts[hh][:, j, :],
                                        in_=pts[hh][:, j, :],
                                        pattern=[[1, 512]],
                                        compare_op=mybir.AluOpType.is_ge,
                                        fill=0.0, base=-r,
                                        channel_multiplier=-1,
                                    )
                        # attended MMs one group late: PE never blocks on
                        # this group's exp -- it still has next scores ready
                        if prev is not None:
                            emit_att(*prev)
                        prev = (kts, pts)
                    emit_att(*prev)
                    for hh in range(2):
                        base = 64 * hh
                        rec = nb.tile([128, 512], f32, tag="rec", name="rec")
                        nc.vector.reciprocal(
                            rec[base + 32 : base + 33, :],
                            att[base + 32 : base + 33, :],
                        )
                        sidx = 2 * qc + hh
                        nc.sync.dma_start(
                            scr_d[sidx : sidx + 1, :],
                            rec[base + 32 : base + 33, :],
                        )
                        bc = nb.tile([128, 512], f32, tag="bc", name="bc")
                        nc.sync.dma_start(
                            bc[base : base + D, :],
                            scr_d[sidx : sidx + 1, :].broadcast_to([D, 512]),
                        )
                        an = nb.tile([128, 512], bf16, tag="an", name="an")
                        nc.vector.tensor_tensor(
                            an[base : base + D, :], att[base : base + D, :],
                            bc[base : base + D, :], OP.mult,
                        )
                        nc.tensor.matmul(
                            po, wo_sb[base : base + D, :],
                            an[base : base + D, :],
                            start=(hh == 0), stop=(hh == 1),
                        )
                    # transpose out^T [64,512] back to row-major and store
                    oT = fb.tile([E, 512], bf16, tag="oT", name="oT")
                    nc.vector.tensor_copy(oT, po)
                    tr = ap_.tile([128, 4, E], bf16, tag="att", name="tr")
                    for i in range(4):
                        nc.tensor.matmul(
                            tr[:, i, :], oT[:, 128 * i : 128 * (i + 1)], idb_sb,
                            is_transpose=True, start=True, stop=True,
                        )
                    ob = fb.tile([128, 4, E], f32, tag="ob", name="ob")
                    nc.vector.tensor_copy(ob, tr)
                    nc.sync.dma_start(
                        out_d[qs, :].rearrange("(c p) e -> p c e", p=128), ob
                    )
    # populate .instr bytes for extended-inst InstISA subclasses (raw Bass
    # does not run this pass; without it walrus fails "ISA wrong length")
    from concourse.library_overlay import lower_extended_insts
    lower_extended_insts(nc)
    if split_waits:  # required for walrus; breaks CoreSim's race detector
        _split_multi_waits(nc, mybir)
    return nc


def _split_multi_waits(nc, mybir):
    """This walrus build accepts at most ONE sync-wait command per
    instruction ("Too many sync wait commands").  Tile emits instructions
    with several waits; hoist all but the last into standalone
    InstEventSemaphore (sequencer wait) instructions on the same engine,
    inserted immediately before."""
    import bass_rust

    uid = [0]
    for f in nc.m.functions:
        for blk in f.blocks:
            insts = list(blk.instructions)
            out = []
            changed = False
            for inst in insts:
                si = inst.sync_info
                waits = list(si.on_wait) if si is not None else []
                if len(waits) > 1:
                    changed = True
                    for w in waits[:-1]:
                        ev = mybir.InstEventSemaphore(
                            name=f"WSPLIT-{uid[0]}", ins=[], outs=[]
                        )
                        uid[0] += 1
                        ev.engine = inst.engine
                        ev.sync_info = bass_rust.SyncInfo(
                            on_wait=[w], on_update=[]
                        )
                        out.append(ev)
                    inst.sync_info = bass_rust.SyncInfo(
                        on_wait=[waits[-1]], on_update=list(si.on_update)
                    )
                out.append(inst)
            if changed:
                blk.instructions = out


def _get_nc(probe=None):
    key = ("nc", probe)
    if key not in _CACHE:
        _CACHE[key] = build_nc(probe)
    return _CACHE[key]


def kernel(x, Wq, Wk, Wv, Wo):
    from concourse.bass_utils import run_bass_kernel_spmd

    x = np.asarray(x, dtype=np.float32)
    Wq, Wk, Wv, Wo = (np.asarray(w, dtype=np.float32) for w in (Wq, Wk, Wv, Wo))

    nc = _get_nc()
    in_maps = [make_core_inputs(x, Wq, Wk, Wv, Wo, c) for c in range(NCORES)]
    res = run_bass_kernel_spmd(nc, in_maps, core_ids=list(range(NCORES)))
    out = np.empty((B, S, E), dtype=np.float32)
    for b in range(B):
        out[b] = res.results[2 * b]["out"] + res.results[2 * b + 1]["out"]
    return out



# revision 13
# speedup vs baseline: 2.4548x; 1.3284x over previous
"""Causal self-attention with RoPE on 8 Trainium2 NeuronCores.

Problem: B=4, S=4096, E=64, H=4 heads x D=16, fp32 in/out.

Sharding: core c handles batch b = c//2 and head-pair hp = c%2 (heads 2*hp,
2*hp+1).  Every core runs the IDENTICAL program (SPMD) -- per-core behavior
comes only from the data (x[b] and per-head weight slices).  Each core
returns the partial output projection sum over its two heads; the host adds
the two partials per batch.

Device algorithm (per core, per head):
  - x^T [64,S] via PE transposes (bf16)
  - K^T,Q^T projections as lhsT.T@x^T (scale 1/sqrt(D) folded into Wq);
    RoPE applied as  rot = proj * cos + proj_shuf * sin  where proj_shuf
    comes from a sign/permuted weight matrix (R@W) -- no cross-partition ops
  - scores computed TRANSPOSED: S^T[k',q] = K^T.T@... (contraction over d=16)
    so softmax normalization folds into the PE: V is augmented with a ones
    column, attended^T[17,q] accumulates over k'-tiles with row 16 = the
    softmax denominator.  Unstable softmax (no max subtraction) is safe here:
    scores ~ N(0,1).
  - causal mask applied post-exp with affine_select (fill 0)
  - normalize with reciprocal + gpsimd partition_broadcast + multiply
  - output projection accumulated over the 2 heads, PE-transposed back to
    row-major, DMA out.
"""

import sys

sys.path.insert(0, "/opt/trn_rl_repo")

import numpy as np
import ml_dtypes

B, S, E, H, D = 4, 4096, 64, 4, 16
NCORES = 8
NKT = S // 128  # 32 k-tiles of 128
NQC = S // 512  # 8 q-chunks of 512
KT_GROUP = 2    # k-tiles per exp batch (2 PSUM banks)

BF16 = ml_dtypes.bfloat16

_CACHE: dict = {}


def _rope_tables():
    # cos/sin[16*hh + d, s] = cos/sin(s * invfreq[d//2]); same for both heads
    pos = np.arange(S, dtype=np.float64)
    pair = np.arange(0, D, 2, dtype=np.float64)  # 0,2,..,14
    inv = 1.0 / (10000.0 ** (pair / D))          # [8]
    ang = pos[None, :] * inv[:, None]            # [8, S]
    cos8, sin8 = np.cos(ang), np.sin(ang)
    cos16 = np.repeat(cos8, 2, axis=0)           # [16, S] rows 2p,2p+1 equal
    sin16 = np.repeat(sin8, 2, axis=0)
    cos32 = np.concatenate([cos16, cos16], axis=0)  # [32, S] both heads
    sin32 = np.concatenate([sin16, sin16], axis=0)
    return cos32.astype(BF16), sin32.astype(BF16)


def _shuffle_rows(w):
    # (R w)[2p] = -w[2p+1], (R w)[2p+1] = w[2p]   (rope partner)
    ws = np.empty_like(w)
    ws[0::2] = -w[1::2]
    ws[1::2] = w[0::2]
    return ws


def make_core_inputs(x, Wq, Wk, Wv, Wo, core):
    """Build the per-core input map (all host-side numpy)."""
    b, hp = core // 2, core % 2
    rs = slice(32 * hp, 32 * hp + 32)  # rows of the 2 heads in W{q,k,v}
    scale = 1.0 / np.sqrt(np.float32(D))

    wq_sel = (Wq[rs] * scale).astype(np.float32)  # [32, 64]
    wk_sel = Wk[rs].astype(np.float32)
    cos32, sin32 = _CACHE.setdefault("rope", _rope_tables())

    def gap48(w32):
        # [32,64] head rows -> [64,48] lhsT with head hh at cols 32*hh+0:16
        out = np.zeros((64, 48), np.float32)
        out[:, 0:16] = w32[0:16].T
        out[:, 32:48] = w32[16:32].T
        return out

    def gap48t(t32):
        out = np.zeros((48, t32.shape[1]), t32.dtype)
        out[0:16] = t32[0:16]
        out[32:48] = t32[16:32]
        return out

    return {
        "xt": np.ascontiguousarray(x[b].T).astype(BF16),              # [64,S]
        "wq": np.ascontiguousarray(gap48(wq_sel)).astype(BF16),       # [64,48]
        "wk": np.ascontiguousarray(gap48(wk_sel)).astype(BF16),
        "wqs": np.ascontiguousarray(gap48(_shuffle_rows(wq_sel))).astype(BF16),
        "wks": np.ascontiguousarray(gap48(_shuffle_rows(wk_sel))).astype(BF16),
        "wv": np.ascontiguousarray(Wv[rs].T).astype(BF16),            # [64,32]
        # wo[d, hh, e] = Wo[e, 16*(2hp+hh)+d]
        "wo": np.ascontiguousarray(
            Wo[:, rs].reshape(E, 2, D).transpose(2, 1, 0)
        ).astype(BF16),                                               # [16,2,64]
        "cost": gap48t(cos32),
        "sint": gap48t(sin32),
        "idt": np.eye(128, dtype=BF16),
    }


def partial_reference(inp):
    """Numpy reference of ONE core's partial output (for testing)."""
    x = inp["xt"].astype(np.float64).T
    cos = inp["cost"].astype(np.float64)[0:16]
    sin = inp["sint"].astype(np.float64)[0:16]
    out = np.zeros((S, E))
    for hh in range(2):
        wq = inp["wq"].astype(np.float64)[:, 32 * hh : 32 * hh + 16]
        wqs = inp["wqs"].astype(np.float64)[:, 32 * hh : 32 * hh + 16]
        wk = inp["wk"].astype(np.float64)[:, 32 * hh : 32 * hh + 16]
        wks = inp["wks"].astype(np.float64)[:, 32 * hh : 32 * hh + 16]
        wv = inp["wv"].astype(np.float64)[:, 16 * hh : 16 * hh + 16]
        wo = inp["wo"].astype(np.float64)[:, hh, :]  # [16, 64]
        q = (x @ wq) * cos.T + (x @ wqs) * sin.T     # [S,16]
        k = (x @ wk) * cos.T + (x @ wks) * sin.T
        v = x @ wv
        s = q @ k.T
        mask = np.tril(np.ones((S, S), dtype=bool))
        p = np.where(mask, np.exp(s), 0.0)
        a = (p @ v) / p.sum(-1, keepdims=True)       # [S,16]
        out += a @ wo
    return out.astype(np.float32)


def build_nc(probe=None, amp=1, split_waits=True, ablate=None):
    """Build the (single, SPMD) Bass program.

    Pipeline: one fused loop over ci = 0..7.  Iteration ci builds x^T chunk
    ci (PE transposes), projects+ropes K^T/Q^T chunk ci, builds V k-tiles
    4ci..4ci+3, then runs causal attention for query chunk qc=ci (which only
    needs K/V up to k-tile 4ci+3).  Projection work for ci+1 gap-fills PE
    stalls during attention of qc=ci.
    """
    import os
    ablate = ablate or os.environ.get("KABLATE") or ()
    import concourse.bass as bass
    import concourse.mybir as mybir
    import concourse.tile as tile

    f32 = mybir.dt.float32
    bf16 = mybir.dt.bfloat16
    AF = mybir.ActivationFunctionType
    OP = mybir.AluOpType

    nc = bass.Bass()
    xt_d = nc.declare_dram_parameter("xt", [E, S], bf16, isOutput=False)
    wq_d = nc.declare_dram_parameter("wq", [E, 48], bf16, isOutput=False)
    wk_d = nc.declare_dram_parameter("wk", [E, 48], bf16, isOutput=False)
    wqs_d = nc.declare_dram_parameter("wqs", [E, 48], bf16, isOutput=False)
    wks_d = nc.declare_dram_parameter("wks", [E, 48], bf16, isOutput=False)
    wv_d = nc.declare_dram_parameter("wv", [E, 32], bf16, isOutput=False)
    wo_d = nc.declare_dram_parameter("wo", [D, 2, E], bf16, isOutput=False)
    cos_d = nc.declare_dram_parameter("cost", [48, S], bf16, isOutput=False)
    sin_d = nc.declare_dram_parameter("sint", [48, S], bf16, isOutput=False)
    idt_d = nc.declare_dram_parameter("idt", [128, 128], bf16, isOutput=False)
    out_d = nc.declare_dram_parameter("out", [S, E], f32, isOutput=True)

    with tile.TileContext(nc) as tc:
        with tc.tile_pool(name="persist", bufs=1) as pp:
            # ---- constants into SBUF ----
            wq_sb = pp.tile([E, 48], bf16, name="wq_sb")
            wk_sb = pp.tile([E, 48], bf16, name="wk_sb")
            wqs_sb = pp.tile([E, 48], bf16, name="wqs_sb")
            wks_sb = pp.tile([E, 48], bf16, name="wks_sb")
            wv_sb = pp.tile([E, 32], bf16, name="wv_sb")
            cos_sb = pp.tile([48, S], bf16, name="cos_sb")
            sin_sb = pp.tile([48, S], bf16, name="sin_sb")
            idt_sb = pp.tile([128, 128], bf16, name="idt_sb")
            for sb, dr in [
                (wq_sb, wq_d), (wk_sb, wk_d), (wqs_sb, wqs_d), (wks_sb, wks_d),
                (wv_sb, wv_d), (cos_sb, cos_d), (sin_sb, sin_d),
                (idt_sb, idt_d),
            ]:
                nc.sync.dma_start(sb, dr[:])
            # wo per head at partitions 64*hh (so the out-projection lhsT
            # shares the contraction partition range with an[64*hh:...])
            wo_sb = pp.tile([128, E], bf16, name="wo_sb")
            for hh in range(2):
                nc.sync.dma_start(wo_sb[64 * hh : 64 * hh + D, :], wo_d[:, hh, :])

            # ---- persistent activations ----
            xT = pp.tile([E, S], bf16, name="xT")
            nc.sync.dma_start(xT, xt_d[:])
            rotK = pp.tile([48, S], bf16, name="rotK")
            rotQ = pp.tile([48, S], bf16, name="rotQ")
            vp = pp.tile([128, NKT, 2, 33], bf16, name="vp")
            nc.vector.memset(vp, 0.0)
            nc.vector.memset(vp[:, :, :, 32:33], 1.0)

            with tc.tile_pool(name="a_pr", bufs=1, space="PSUM") as pr, \
                 tc.tile_pool(name="a_ps", bufs=1, space="PSUM") as sp, \
                 tc.tile_pool(name="a_att", bufs=1, space="PSUM") as ap_, \
                 tc.tile_pool(name="a_po", bufs=1, space="PSUM") as op_, \
                 tc.tile_pool(name="a_p", bufs=3) as pb_, \
                 tc.tile_pool(name="a_n", bufs=2) as nb, \
                 tc.tile_pool(name="a_f", bufs=2) as fb:
              for _rep in range(amp):
                for ci in range(NQC + 1):
                  if ci < NQC:
                    cs = slice(512 * ci, 512 * (ci + 1))
                    # -- projections + rope, chunk ci --
                    for wmat, wsh, rot, nm in (
                        (wk_sb, wks_sb, rotK, "k"),
                        (wq_sb, wqs_sb, rotQ, "q"),
                    ):
                        pa = pr.tile([48, 512], f32, tag="pa", name=f"pa{nm}")
                        nc.tensor.matmul(pa, wmat, xT[:, cs], start=True, stop=True)
                        pb = pr.tile([48, 512], f32, tag="pb", name=f"pb{nm}")
                        nc.tensor.matmul(pb, wsh, xT[:, cs], start=True, stop=True)
                        t1 = nb.tile([48, 512], bf16, tag=f"t1{nm}", name=f"t1{nm}")
                        nc.vector.tensor_tensor(t1, pa, cos_sb[:, cs], OP.mult)
                        t2 = nb.tile([48, 512], bf16, tag=f"t2{nm}", name=f"t2{nm}")
                        nc.vector.tensor_tensor(t2, pb, sin_sb[:, cs], OP.mult)
                        nc.vector.tensor_tensor(rot[:, cs], t1, t2, OP.add)
                    # -- V k-tiles of chunk ci --
                    for ii in range(4):
                        i = 4 * ci + ii
                        pv = pr.tile([128, 32], f32, tag=("pa", "pb")[ii % 2],
                                     name="pv")
                        nc.tensor.matmul(
                            pv, xT[:, 128 * i : 128 * (i + 1)], wv_sb,
                            start=True, stop=True,
                        )
                        nc.vector.tensor_copy(
                            vp[:, i, :, 0:D],
                            pv.rearrange("p (h d) -> p h d", h=2),
                        )
                  # -- causal attention, one chunk behind projection --
                  qc = ci - 1
                  if qc >= 0:
                    nk = 4 * qc + 4
                    qs = slice(512 * qc, 512 * (qc + 1))
                    att = ap_.tile([128, 512], f32, tag="att", name="att")
                    # both heads accumulate in one bank (rows 64*hh..+33).
                    # start=True would clear the whole bank's has_written
                    # bits and corrupt the other head's region, so memset
                    # the values once and accumulate with start=False
                    # (accumulate-onto-0 == overwrite, either bit state).
                    nc.vector.memset(att, 0.0)
                    def emit_att(kts_, pts_):
                        for j, kt in enumerate(kts_):
                            for hh in range(2):
                                nc.tensor.matmul(
                                    att[64 * hh : 64 * hh + 33, :],
                                    vp[:, kt, hh, :], pts_[hh][:, j, :],
                                    start=False, stop=(kt == nk - 1),
                                    skip_group_check=True,
                                    tile_position=(0, 64 * hh),
                                )

                    prev = None
                    for g0 in range(0, nk, KT_GROUP):
                        kts = list(range(g0, min(g0 + KT_GROUP, nk)))
                        pss, pts = [], []
                        for hh in range(2):
                            beta = 32 * hh
                            ps = sp.tile([128, KT_GROUP, 512], f32,
                                         tag=f"s{hh}", name=f"ps{hh}")
                            pss.append(ps)
                            for j, kt in enumerate(kts):
                                nc.tensor.matmul(
                                    ps[:, j, :],
                                    rotK[beta : beta + D,
                                         128 * kt : 128 * (kt + 1)],
                                    rotQ[beta : beta + D, qs],
                                    start=True, stop=True,
                                )
                        for hh in range(2):
                            pt = pb_.tile([128, KT_GROUP, 512], bf16,
                                          tag=f"p{hh}", name=f"pt{hh}")
                            pts.append(pt)
                            nc.scalar.activation(
                                pt[:, 0 : len(kts), :],
                                pss[hh][:, 0 : len(kts), :], AF.Exp,
                            )
                        for j, kt in enumerate(kts):
                            for hh in range(2):
                                r = 128 * kt - 512 * qc
                                if r >= 0 and "noaffine" not in ablate:
                                    nc.gpsimd.affine_select(
                                        out=pts[hh][:, j, :],
                                        in_=pts[hh][:, j, :],
                                        pattern=[[1, 512]],
                                        compare_op=mybir.AluOpType.is_ge,
                                        fill=0.0, base=-r,
                                        channel_multiplier=-1,
                                    )
                        # attended MMs one group late: PE never blocks on
                        # this group's exp -- it still has next scores ready
                        if prev is not None:
                            emit_att(*prev)
                        prev = (kts, pts)
                    emit_att(*prev)
                    # ---- tail: normalize + output projection, row-major ----
                    attS = fb.tile([128, 512], bf16, tag="attS", name="attS")
                    nc.vector.tensor_copy(attS, att)
                    attT = op_.tile([128, 4, 128], bf16, tag="po", name="attT")
                    for c in range(4):
                        nc.tensor.matmul(
                            attT[:, c, :], attS[:, 128 * c : 128 * (c + 1)],
                            idt_sb, is_transpose=True, start=True, stop=True,
                        )
                    rec = nb.tile([128, 4, 2], f32, tag="rec", name="rec")
                    nc.vector.reciprocal(rec, attT[:, :, 32:97:64])
                    out_sb = fb.tile([128, 4, E], f32, tag="ob", name="ob")
                    for c in range(4):
                        # two heads' projections drain into DIFFERENT banks
                        # (concurrent row-tiled matmuls into one bank fault)
                        ops0 = ap_.tile([128, E], f32, tag="att", name="ops0")
                        ops1 = op_.tile([128, E], f32, tag="po", name="ops1")
                        for hh, opst in ((0, ops0), (1, ops1)):
                            base = 64 * hh
                            nc.tensor.matmul(
                                opst,
                                attS[base : base + D, 128 * c : 128 * (c + 1)],
                                wo_sb[base : base + D, :],
                                start=True, stop=True,
                                tile_position=(base, 0),
                            )
                        u0 = nb.tile([128, E], f32, tag="u0", name="u0")
                        nc.vector.tensor_scalar(
                            u0, ops0, rec[:, c, 0:1], None, OP.mult,
                        )
                        nc.vector.scalar_tensor_tensor(
                            out_sb[:, c, :], ops1, rec[:, c, 1:2], u0,
                            OP.mult, OP.add,
                        )
                    nc.sync.dma_start(
                        out_d[qs, :].rearrange("(c p) e -> p c e", p=128), out_sb
                    )
    # populate .instr bytes for extended-inst InstISA subclasses (raw Bass
    # does not run this pass; without it walrus fails "ISA wrong length")
    from concourse.library_overlay import lower_extended_insts
    lower_extended_insts(nc)
    if split_waits:  # required for walrus; breaks CoreSim's race detector
        _split_multi_waits(nc, mybir)
    return nc


def _split_multi_waits(nc, mybir):
    """This walrus build accepts at most ONE sync-wait command per
    instruction ("Too many sync wait commands").  Tile emits instructions
    with several waits; hoist all but the last into standalone
    InstEventSemaphore (sequencer wait) instructions on the same engine,
    inserted immediately before."""
    import bass_rust

    uid = [0]
    for f in nc.m.functions:
        for blk in f.blocks:
            insts = list(blk.instructions)
            out = []
            changed = False
            for inst in insts:
                si = inst.sync_info
                waits = list(si.on_wait) if si is not None else []
                if len(waits) > 1:
                    changed = True
                    for w in waits[:-1]:
                        ev = mybir.InstEventSemaphore(
                            name=f"WSPLIT-{uid[0]}", ins=[], outs=[]
                        )
                        uid[0] += 1
                        ev.engine = inst.engine
                        ev.sync_info = bass_rust.SyncInfo(
                            on_wait=[w], on_update=[]
                        )
                        out.append(ev)
                    inst.sync_info = bass_rust.SyncInfo(
                        on_wait=[waits[-1]], on_update=list(si.on_update)
                    )
                out.append(inst)
            if changed:
                blk.instructions = out


def _get_nc(probe=None):
    key = ("nc", probe)
    if key not in _CACHE:
        _CACHE[key] = build_nc(probe)
    return _CACHE[key]


def kernel(x, Wq, Wk, Wv, Wo):
    from concourse.bass_utils import run_bass_kernel_spmd

    x = np.asarray(x, dtype=np.float32)
    Wq, Wk, Wv, Wo = (np.asarray(w, dtype=np.float32) for w in (Wq, Wk, Wv, Wo))

    nc = _get_nc()
    in_maps = [make_core_inputs(x, Wq, Wk, Wv, Wo, c) for c in range(NCORES)]
    res = run_bass_kernel_spmd(nc, in_maps, core_ids=list(range(NCORES)))
    out = np.empty((B, S, E), dtype=np.float32)
    for b in range(B):
        out[b] = res.results[2 * b]["out"] + res.results[2 * b + 1]["out"]
    return out

